# revision 38
# baseline (speedup 1.0000x reference)
"""Multi-head attention (S=4096, E=1024, H=16 heads, D=128) on 8 TRN2 NeuronCores.

Sharding: tensor-parallel over heads (2 heads/core) for QKV projections and
attention; AllToAll re-shards attention output to sequence-parallel for the
output projection (each core computes its 512-row slice of the output).

All large matmuls run as fp32r (11-bit-mantissa fp32, full PE rate at N>=256).
Softmax skips max-subtraction (|scaled scores| < ~10, exp is fp32-safe);
denominators via DVE adds + one fp32 ones-matmul for the cross-partition
reduction + broadcast.
"""

import os
from contextlib import ExitStack

import numpy as np

import concourse.bacc as bacc
import concourse.mybir as mybir
import concourse.tile as tile
from concourse.bass_utils import run_bass_kernel_spmd

S, E, H, DH = 4096, 1024, 16, 128
NCORES = 8
HPC = H // NCORES  # heads per core = 2
SC = S // NCORES  # seq rows per core for output projection = 512
NB = S // 512  # qrow blocks = 8
NKT = S // 128  # key tiles = 32
NE = E // 128  # embed chunks = 8
SCALE = float(1.0 / np.sqrt(np.float32(DH)))

F32 = mybir.dt.float32
F32R = mybir.dt.float32r
F16 = mybir.dt.float16

EXPP_BUFS = 22  # SBUF bufs for exp(P^T) tiles
GROUP = 2  # key-tiles per exp activation op
EXP_BIAS = -2.0  # exp(s*scale + b): uniform shift cancels in softmax,
# keeps fp16 P well under overflow


def _r32r(x):
    """Round fp32 ndarray to fp32r (round-to-nearest, 11-bit mantissa)."""
    b = np.ascontiguousarray(np.asarray(x, np.float32)).view(np.uint32)
    out = ((b + np.uint32(1 << 11)) & np.uint32(0xFFFFF000)).view(np.float32)
    return np.ascontiguousarray(out)


def _positional_encoding():
    pos = np.arange(S, dtype=np.float32)[:, None]
    expo = np.arange(0, E, 2, dtype=np.float32)
    with np.errstate(over="ignore"):
        denominator = np.float32(1.0) / (
            np.power(np.float32(10000.0), expo) / np.float32(E)
        )
    ang = pos * denominator[None, :]
    pe = np.stack([np.sin(ang), np.cos(ang)], axis=-1).reshape(S, E)
    return pe.astype(np.float32)


def _build(collective=True):
    nc = bacc.Bacc(None, num_devices=NCORES)

    xpT = nc.dram_tensor("xpT", [E, S], F32R, kind="ExternalInput")
    wq = nc.dram_tensor("wq", [HPC, E, DH], F32R, kind="ExternalInput")
    wk = nc.dram_tensor("wk", [HPC, E, DH], F32R, kind="ExternalInput")
    wv2 = nc.dram_tensor("wv2", [E, HPC * DH], F32R, kind="ExternalInput")
    wo = nc.dram_tensor("wo", [H * DH, E], F32R, kind="ExternalInput")
    bq2 = nc.dram_tensor("bq2", [HPC, DH, 1], F32, kind="ExternalInput")
    bk2 = nc.dram_tensor("bk2", [HPC, DH, 1], F32, kind="ExternalInput")
    bv2 = nc.dram_tensor("bv2", [1, HPC * DH], F32, kind="ExternalInput")
    bo = nc.dram_tensor("bo", [1, E], F32, kind="ExternalInput")
    y = nc.dram_tensor("y", [SC, E], F32, kind="ExternalOutput")

    with tile.TileContext(nc) as tc, ExitStack() as es:
        cpool = es.enter_context(tc.tile_pool(name="cpool", bufs=1))

        # ---- constants ----
        ones_row = cpool.tile([1, 512], F32)
        nc.vector.memset(ones_row[:], 1.0)
        ones128 = cpool.tile([128, 128], F32)
        nc.vector.memset(ones128[:], 1.0)
        expbias = cpool.tile([128, 1], F32)
        nc.vector.memset(expbias[:], EXP_BIAS)

        bqt = []
        bkt = []
        for h in range(HPC):
            t1 = cpool.tile([DH, 1], F32, name=f"bqt{h}")
            nc.sync.dma_start(t1[:], bq2[h])
            bqt.append(t1)
            t2 = cpool.tile([DH, 1], F32, name=f"bkt{h}")
            nc.sync.dma_start(t2[:], bk2[h])
            bkt.append(t2)

        bv_row = cpool.tile([1, HPC * DH], F32)
        nc.sync.dma_start(bv_row[:], bv2[:])
        bo_row = cpool.tile([1, E], F32)
        nc.sync.dma_start(bo_row[:], bo[:])

        # broadcast bias rows across partitions via K=1 fp32 matmuls
        with tc.tile_pool(name="cpsum", bufs=1, space="PSUM") as cpsum:
            pbv = cpsum.tile([128, HPC * DH], F32)
            nc.tensor.matmul(
                pbv[:], ones_row[:, 0:128], bv_row[:], start=True, stop=True
            )
            bv_bcast = cpool.tile([128, HPC * DH], F32)
            nc.scalar.copy(bv_bcast[:], pbv[:])

            pbo = cpsum.tile([128, E], F32)
            for nh in range(2):
                nc.tensor.matmul(
                    pbo[:, nh * 512 : (nh + 1) * 512],
                    ones_row[:, 0:128],
                    bo_row[:, nh * 512 : (nh + 1) * 512],
                    start=True,
                    stop=True,
                )
            bo_bcast = cpool.tile([128, E], F32)
            nc.scalar.copy(bo_bcast[:], pbo[:])

        # ---- persistent SBUF for q^T, k^T (per head) and packed v ----
        qkv_pool_cm = tc.tile_pool(name="qkv", bufs=1)
        qkv_pool = qkv_pool_cm.__enter__()
        qT = [qkv_pool.tile([DH, S], F32R, name=f"qT{h}") for h in range(HPC)]
        kT = [qkv_pool.tile([DH, S], F32R, name=f"kT{h}") for h in range(HPC)]
        v_sb = qkv_pool.tile([128, NKT * HPC * DH], F16, name="v_sb")

        # pools that span projection AND attention phases
        pmisc_cm = tc.tile_pool(name="pmisc", bufs=2, space="PSUM")
        pmisc = pmisc_cm.__enter__()  # qk accumulators + denominators
        xstrip_cm = tc.tile_pool(name="xstrip", bufs=2)
        xstrip = xstrip_cm.__enter__()
        wpool1_cm = tc.tile_pool(name="wpool1", bufs=1)
        wpool1 = wpool1_cm.__enter__()  # head-1 q/k weights, used mid-attention
        wq1_sb = wpool1.tile([128, NE * DH], F32R, name="wq1_sb")
        wk1_sb = wpool1.tile([128, NE * DH], F32R, name="wk1_sb")
        wq1_t = [wq1_sb[:, e * DH : (e + 1) * DH] for e in range(NE)]
        wk1_t = [wk1_sb[:, e * DH : (e + 1) * DH] for e in range(NE)]

        def load_strip(s):
            """One batched DMA for a full [E, 512] strip of xpT; returns the
            strip tile whose column block e*512:(e+1)*512 is E-chunk e."""
            t = xstrip.tile([128, NE * 512], F32R, tag="xs", name=f"xs{s}")
            nc.sync.dma_start(
                t[:].rearrange("p (e c) -> p e c", e=NE),
                xpT[:, s * 512 : (s + 1) * 512].rearrange(
                    "(e p) c -> p e c", p=128
                ),
            )
            return [t[:, e * 512 : (e + 1) * 512] for e in range(NE)]

        # ---- phase A: v (both heads) + head-0 q/k projections ----
        with (
            tc.tile_pool(name="wpool0", bufs=1) as wpool0,
            tc.tile_pool(name="pv", bufs=4, space="PSUM") as pv,
        ):
            wq0_sb = wpool0.tile([128, NE * DH], F32R, name="wq0_sb")
            wk0_sb = wpool0.tile([128, NE * DH], F32R, name="wk0_sb")
            wv_sb2 = wpool0.tile([128, NE * HPC * DH], F32R, name="wv_sb2")
            wq0_t = [wq0_sb[:, e * DH : (e + 1) * DH] for e in range(NE)]
            wk0_t = [wk0_sb[:, e * DH : (e + 1) * DH] for e in range(NE)]
            wv_t = [
                wv_sb2[:, e * HPC * DH : (e + 1) * HPC * DH] for e in range(NE)
            ]
            xs0 = load_strip(0)  # first strip ahead of weight loads
            for dst, src in (
                (wq0_sb, wq[0]),
                (wk0_sb, wk[0]),
                (wv_sb2, wv2[:]),
                (wq1_sb, wq[1]),
                (wk1_sb, wk[1]),
            ):
                nc.sync.dma_start(
                    dst[:].rearrange("p (e d) -> p e d", e=NE),
                    src.rearrange("(e p) d -> p e d", p=128),
                )

            for s in range(NB):
                xs = xs0 if s == 0 else load_strip(s)
                for w_t, bt, dstT in (
                    (wq0_t, bqt[0], qT[0]),
                    (wk0_t, bkt[0], kT[0]),
                ):
                    pq = pmisc.tile([128, 512], F32, tag="pqdn", name=f"pq{s}")
                    for e in range(NE):
                        nc.tensor.matmul(
                            pq[:],
                            w_t[e][:],
                            xs[e][:],
                            start=(e == 0),
                            stop=(e == NE - 1),
                        )
                    nc.scalar.activation(
                        dstT[:, s * 512 : (s + 1) * 512],
                        pq[:],
                        mybir.ActivationFunctionType.Identity,
                        bias=bt[:],
                    )
                for st in range(4):
                    pvt = pv.tile([128, HPC * DH], F32, tag="pv", name=f"pv{s}{st}")
                    for e in range(NE):
                        nc.tensor.matmul(
                            pvt[:],
                            xs[e][:, st * 128 : (st + 1) * 128],
                            wv_t[e][:],
                            start=(e == 0),
                            stop=(e == NE - 1),
                        )
                    kt_idx = s * 4 + st
                    nc.vector.tensor_add(
                        v_sb[
                            :, kt_idx * HPC * DH : (kt_idx + 1) * HPC * DH
                        ],
                        pvt[:],
                        bv_bcast[:],
                    )

        # ---- attention phase (per head), A2A per head ----
        dram = es.enter_context(tc.tile_pool(name="dram", bufs=1, space="DRAM"))
        a2a_in = [
            dram.tile([NCORES, 128, 512], F32R, name=f"a2a_in{h}")
            for h in range(HPC)
        ]
        a2a_out = [
            dram.tile([NCORES, 128, 512], F32R, name=f"a2a_out{h}")
            for h in range(HPC)
        ]

        with (
            tc.tile_pool(name="ptpool", bufs=EXPP_BUFS) as ptpool,
            tc.tile_pool(name="accp", bufs=2) as accp,
            tc.tile_pool(name="rbp", bufs=2) as rbp,
            tc.tile_pool(name="anp", bufs=2) as anp,
            tc.tile_pool(name="psc", bufs=2, space="PSUM") as psc,
            tc.tile_pool(name="patt", bufs=2, space="PSUM") as patt,
        ):
            for h in range(HPC):
                for b in range(NB):
                    qs = qT[h][:, b * 512 : (b + 1) * 512]
                    acc = accp.tile([128, 512], F32, tag="acc", name=f"acc{h}{b}")
                    attp = patt.tile([128, 512], F32, tag="att", name=f"att{h}{b}")
                    if h == 0:
                        # head-1 q/k projection rides in PE slack of the
                        # ACT-bound attention steady state (strip s == b)
                        xs1 = load_strip(b)
                        p1 = {}
                    eps = []
                    for g in range(NKT // GROUP):
                        sc = psc.tile(
                            [128, GROUP * 512], F32, tag="sc", name=f"sc{h}{b}{g}"
                        )
                        for j in range(GROUP):
                            kt = GROUP * g + j
                            nc.tensor.matmul(
                                sc[:, j * 512 : (j + 1) * 512],
                                kT[h][:, kt * 128 : (kt + 1) * 128],
                                qs,
                                start=True,
                                stop=True,
                            )
                        if h == 0:
                            e1 = g % NE
                            w_t = wq1_t if g < NE else wk1_t
                            if e1 == 0:
                                p1["t"] = pmisc.tile(
                                    [128, 512], F32, tag="pqdn", name=f"p1{b}{g}"
                                )
                            nc.tensor.matmul(
                                p1["t"][:],
                                w_t[e1][:],
                                xs1[e1][:],
                                start=(e1 == 0),
                                stop=(e1 == NE - 1),
                            )
                        ep = ptpool.tile(
                            [128, GROUP * 512], F16, tag="pt", name=f"ep{h}{b}{g}"
                        )
                        nc.scalar.activation(
                            ep[:],
                            sc[:],
                            mybir.ActivationFunctionType.Exp,
                            scale=SCALE,
                            bias=expbias[:],
                        )
                        eps.append(ep)
                        for j in range(GROUP):
                            kt = GROUP * g + j
                            nc.tensor.matmul(
                                attp[:],
                                v_sb[
                                    :,
                                    kt * HPC * DH
                                    + h * DH : kt * HPC * DH
                                    + (h + 1) * DH,
                                ],
                                ep[:, j * 512 : (j + 1) * 512],
                                start=(kt == 0),
                                stop=(kt == NKT - 1),
                            )
                        # level-1 partial: fold the pair into the low half (fp16, 2x)
                        nc.vector.tensor_add(
                            ep[:, 0:512], ep[:, 0:512], ep[:, 512:1024]
                        )
                        # interleave higher tree levels as their inputs complete
                        if g % 2 == 1:
                            j = g // 2
                            nc.vector.tensor_add(
                                eps[2 * j][:, 0:512],
                                eps[2 * j][:, 0:512],
                                eps[2 * j + 1][:, 0:512],
                            )
                        if g % 4 == 3:
                            j = g // 4
                            nc.vector.tensor_add(
                                eps[4 * j + 1][:].bitcast(F32),
                                eps[4 * j][:, 0:512],
                                eps[4 * j + 2][:, 0:512],
                            )
                        if g % 8 == 7:
                            j = g // 8
                            nc.vector.tensor_add(
                                eps[8 * j + 3][:].bitcast(F32),
                                eps[8 * j + 1][:].bitcast(F32),
                                eps[8 * j + 5][:].bitcast(F32),
                            )
                        if h == 0 and e1 == NE - 1:
                            # evacuate finished head-1 projection via DVE
                            dstT, bt = (
                                (qT[1], bqt[1]) if g < NE else (kT[1], bkt[1])
                            )
                            nc.vector.tensor_scalar_add(
                                dstT[:, b * 512 : (b + 1) * 512],
                                p1["t"][:],
                                bt[:],
                            )
                    # final tree level (lower levels interleaved in the g-loop)
                    nc.vector.tensor_add(
                        acc[:], eps[3][:].bitcast(F32), eps[11][:].bitcast(F32)
                    )
                    dnb = pmisc.tile([128, 512], F32, tag="pqdn", name=f"dn{h}{b}")
                    nc.tensor.matmul(
                        dnb[:], ones128[:], acc[:], start=True, stop=True
                    )
                    rb = rbp.tile([128, 512], F32, tag="rb", name=f"rb{h}{b}")
                    nc.vector.reciprocal(rb[:], dnb[:])
                    an = anp.tile([128, 512], F32R, tag="an", name=f"an{h}{b}")
                    nc.vector.tensor_mul(an[:], attp[:], rb[:])
                    nc.sync.dma_start(a2a_in[h][b], an[:])
                if collective:
                    nc.gpsimd.collective_compute(
                        "AllToAll",
                        mybir.AluOpType.bypass,
                        replica_groups=[list(range(NCORES))],
                        ins=[a2a_in[h][:]],
                        outs=[a2a_out[h][:]],
                    )

        wpool1_cm.__exit__(None, None, None)
        xstrip_cm.__exit__(None, None, None)
        qkv_pool_cm.__exit__(None, None, None)

        # ---- output projection on this core's 512-row slice ----
        with (
            tc.tile_pool(name="opool", bufs=1) as opool,
            tc.tile_pool(name="obp", bufs=2) as obp,
            tc.tile_pool(name="ppo", bufs=2, space="PSUM") as ppo,
        ):
            a2a_src = a2a_out if collective else a2a_in
            aT_sb = []
            for h in range(HPC):
                t = opool.tile([128, NCORES * 512], F32R, name=f"aT{h}")
                nc.sync.dma_start(
                    t[:].rearrange("p (i c) -> p i c", i=NCORES),
                    a2a_src[h][:].rearrange("i p c -> p i c"),
                )
                aT_sb.append(t)
            wo_sb = []
            for half in range(2):
                t = opool.tile([128, 8 * E], F32R, name=f"woT{half}")
                nc.sync.dma_start(
                    t[:].rearrange("p (g d) -> p g d", g=8),
                    wo[half * 1024 : (half + 1) * 1024, :].rearrange(
                        "(g p) d -> p g d", p=128
                    ),
                )
                wo_sb.append(t)

            order = [(i, 0) for i in range(NCORES)] + [
                (i, 1) for i in range(NCORES)
            ]
            for rt in range(4):
                po = ppo.tile([128, E], F32, tag="po", name=f"po{rt}")
                for idx, (i, h) in enumerate(order):
                    g16 = 2 * i + h
                    for nh in range(2):
                        nc.tensor.matmul(
                            po[:, nh * 512 : (nh + 1) * 512],
                            aT_sb[h][:, i * 512 + rt * 128 : i * 512 + (rt + 1) * 128],
                            wo_sb[g16 // 8][
                                :,
                                (g16 % 8) * E
                                + nh * 512 : (g16 % 8) * E
                                + (nh + 1) * 512,
                            ],
                            start=(idx == 0),
                            stop=(idx == len(order) - 1),
                        )
                ob = obp.tile([128, E], F32, tag="ob", name=f"ob{rt}")
                nc.vector.tensor_add(ob[:], po[:], bo_bcast[:])
                nc.sync.dma_start(y[rt * 128 : (rt + 1) * 128, :], ob[:])

        pmisc_cm.__exit__(None, None, None)

    nc.compile()
    return nc


_NC = None


def _get_nc():
    global _NC
    if _NC is None:
        _NC = _build()
    return _NC


def make_in_maps(x, Wq, bq, Wk, bk, Wv, bv, Wo, bo):
    pe = _positional_encoding()
    xp = (np.asarray(x, np.float32) + pe).astype(np.float32)
    xpT = _r32r(xp.T)
    wo_full = _r32r(Wo)
    bo_r = np.ascontiguousarray(np.asarray(bo, np.float32).reshape(1, E))
    in_maps = []
    for c in range(NCORES):
        hs = slice(HPC * c, HPC * (c + 1))
        in_maps.append(
            {
                "xpT": xpT,
                "wq": _r32r(Wq[hs]),
                "wk": _r32r(Wk[hs]),
                "wv2": _r32r(
                    np.concatenate([Wv[HPC * c + j] for j in range(HPC)], axis=1)
                ),
                "wo": wo_full,
                "bq2": np.ascontiguousarray(
                    np.asarray(bq[hs], np.float32).reshape(HPC, DH, 1)
                ),
                "bk2": np.ascontiguousarray(
                    np.asarray(bk[hs], np.float32).reshape(HPC, DH, 1)
                ),
                "bv2": np.ascontiguousarray(
                    np.concatenate(
                        [np.asarray(bv[HPC * c + j], np.float32) for j in range(HPC)]
                    ).reshape(1, HPC * DH)
                ),
                "bo": bo_r,
            }
        )
    return in_maps


def kernel(x, Wq, bq, Wk, bk, Wv, bv, Wo, bo, _trace=False, _trace_kwargs=None):
    nc = _get_nc()
    in_maps = make_in_maps(x, Wq, bq, Wk, bk, Wv, bv, Wo, bo)
    res = run_bass_kernel_spmd(
        nc,
        in_maps,
        list(range(NCORES)),
        trace=_trace,
        **(_trace_kwargs or {}),
    )
    out = np.concatenate([res.results[c]["y"] for c in range(NCORES)], axis=0)
    if _trace:
        kernel.last_results = res
    return out
